# revision 20
# baseline (speedup 1.0000x reference)
"""Bass/Trainium2 kernel for BestMatchDistance.

ref: sim[b,q,s] = sum_d q[b,d,q]*s[b,d,s]; out[b] = mean_q max_s sim.

Sharding: batch dim B=64 split across 8 cores (8 batches/core), pure data
parallel. Inputs are cast to bf16 on the host (full-rate PE, half DMA).

Per (batch, 128-query tile): the [128, 2048] sim row = 4 bf16 matmuls
(K=64, N=512) K-packed 2-up onto PE row-groups 0-63 / 64-127 (query data is
duplicated to both partition halves, support is split), so the two groups'
matmuls run concurrently. a-half sims land in PSUM tile A, b-half sims in
PSUM tile B (separate pools; B is consumed early by ScalarE so it gets 1
buffer, A is held until the scan so it gets 3 — deepening the pipeline).

Evacuation: ScalarE copies B to SBUF bf16 (no drain penalty on ACT); the
DVE folds {A, copy(B)} with running-max tensor_tensor_scans whose stride-0
output APs leave each final state in pmax (one 1024-position scan per
q-tile; finer splits measured slower on HW — per-op overhead exceeds the
drain savings).

Mean over queries = reduce_max over the SPLIT partials, reduce_sum over
tiles, ones-vector matmul over partitions, scaled by 1/NQ.
"""

import numpy as np

B, D, NQ, NS = 64, 64, 2048, 2048
N_CORES = 8
BPC = B // N_CORES  # batches per core
HNS = NS // 2  # support cols per PE row-group
SPLIT = 1  # independent sub-scans per q-tile (1 measured best on HW)

_cache = {}


def _emit_body(nc, mybir, q_d, s_d, o_d, ones, rall, pools, rep=0,
               parts=3, split=SPLIT):
    DO_MM = parts & 1
    DO_EVAC = parts & 2
    f32 = mybir.dt.float32
    bf16 = mybir.dt.bfloat16
    fmax = mybir.AluOpType.max
    X = mybir.AxisListType.X
    qp, sp, pa, pb, scp, rp, finp = pools

    n_qt = NQ // 128  # 16 q-tiles per batch
    W = HNS // max(split, 1)  # positions per sub-scan

    for b in range(BPC):
        qt = qp.tile([128, NQ], bf16, tag="q", name=f"q{rep}_{b}")
        nc.sync.dma_start(out=qt[0:64, :], in_=q_d[b])
        nc.sync.dma_start(out=qt[64:128, :], in_=q_d[b])
        st = sp.tile([128, HNS], bf16, tag="s", name=f"s{rep}_{b}")
        nc.sync.dma_start(out=st[0:64, :], in_=s_d[b][:, 0:HNS])
        nc.sync.dma_start(out=st[64:128, :], in_=s_d[b][:, HNS:NS])

        pmax = rp.tile([128, n_qt, max(split, 1)], f32, tag="pmax",
                       name=f"pm{rep}_{b}")
        if not (DO_MM and DO_EVAC):
            nc.vector.memset(pmax[:], 0.0)
        scr = None
        if split == 0:
            # split=0: scans write a real stride-1 output; the row max is
            # extracted from the last scan position afterwards.
            scr = rp.tile([128, n_qt, HNS], bf16, tag="scr", name=f"sc{rep}_{b}")

        for i in range(n_qt):
            lhs0 = qt[0:64, i * 128 : (i + 1) * 128]
            lhs1 = qt[64:128, i * 128 : (i + 1) * 128]
            A = pa.tile([128, HNS], f32, tag="pa", name=f"A{rep}_{b}_{i}")
            Bt = pb.tile([128, HNS], f32, tag="pb", name=f"B{rep}_{b}_{i}")
            if not DO_MM:
                continue
            for j in range(2):
                sl = slice(j * 512, (j + 1) * 512)
                nc.tensor.matmul(A[:, sl], lhsT=lhs0, rhs=st[0:64, sl],
                                 start=True, stop=True)
                nc.tensor.matmul(Bt[:, sl], lhsT=lhs1, rhs=st[64:128, sl],
                                 start=True, stop=True, tile_position=(64, 0))
            if DO_EVAC:
                bh = scp.tile([128, HNS], bf16, tag="bh",
                              name=f"bh{rep}_{b}_{i}")
                nc.scalar.copy(out=bh[:], in_=Bt[:])
                if split == 0:
                    nc.vector.tensor_tensor_scan(
                        out=scr[:, i, :], data0=A[:], data1=bh[:],
                        initial=-1e30, op0=fmax, op1=fmax)
                else:
                    for k in range(split):
                        ks = slice(k * W, (k + 1) * W)
                        nc.vector.tensor_tensor_scan(
                            out=pmax[:, i, k : k + 1].broadcast_to([128, W]),
                            data0=A[:, ks], data1=bh[:, ks], initial=-1e30,
                            op0=fmax, op1=fmax)

        # batch-end reduce stays on the DVE: routing it through the ACT
        # accumulator measured 90us slower (the accum op depends on all 16
        # scans and head-of-line-blocks the ACT FIFO queue).
        if split == 0 and DO_MM and DO_EVAC:
            nc.vector.reduce_sum(rall[:, b : b + 1],
                                 scr[:, :, HNS - 1 : HNS], axis=X)
        elif split == 1:
            # pmax is [128, n_qt, 1]: one XY-reduce sums the tile maxes.
            nc.vector.reduce_sum(rall[:, b : b + 1], pmax[:],
                                 axis=mybir.AxisListType.XY)
        else:
            pm2 = rp.tile([128, n_qt], f32, tag="pm2", name=f"pm2{rep}_{b}")
            nc.vector.reduce_max(pm2[:], pmax[:], axis=X)
            nc.vector.reduce_sum(rall[:, b : b + 1], pm2[:], axis=X)

    pf = pa.tile([128, HNS], f32, tag="pa", name=f"pf{rep}")
    nc.tensor.matmul(pf[0:1, 0:BPC], lhsT=ones[:], rhs=rall[:],
                     start=True, stop=True)
    ob = finp.tile([1, BPC], f32, tag="ob", name=f"ob{rep}")
    nc.scalar.mul(ob[:], pf[0:1, 0:BPC], 1.0 / NQ)
    nc.sync.dma_start(out=o_d[:], in_=ob[:])


def _build(loop_reps=None, parts=3, split=SPLIT, bufs_a=3, bufs_b=1):
    import concourse.bacc as bacc
    import concourse.mybir as mybir
    import concourse.tile as tile

    f32 = mybir.dt.float32
    bf16 = mybir.dt.bfloat16

    nc = bacc.Bacc("TRN2", target_bir_lowering=False, debug=False)
    q_d = nc.dram_tensor("q", [BPC, D, NQ], bf16, kind="ExternalInput").ap()
    s_d = nc.dram_tensor("s", [BPC, D, NS], bf16, kind="ExternalInput").ap()
    o_d = nc.dram_tensor("o", [1, BPC], f32, kind="ExternalOutput").ap()

    with tile.TileContext(nc) as tc:
        with (
            tc.tile_pool(name="qp", bufs=3) as qp,
            tc.tile_pool(name="sp", bufs=3) as sp,
            tc.tile_pool(name="pa", bufs=bufs_a, space="PSUM") as pa,
            tc.tile_pool(name="pb", bufs=bufs_b, space="PSUM") as pb,
            tc.tile_pool(name="scp", bufs=4) as scp,
            tc.tile_pool(name="rp", bufs=3) as rp,
            tc.tile_pool(name="fin", bufs=1) as finp,
        ):
            ones = finp.tile([128, 1], f32, tag="ones")
            nc.vector.memset(ones[:], 1.0)
            rall = finp.tile([128, BPC], f32, tag="rall")
            pools = (qp, sp, pa, pb, scp, rp, finp)

            if loop_reps is None:
                _emit_body(nc, mybir, q_d, s_d, o_d, ones, rall, pools,
                           parts=parts, split=split)
            else:
                with tc.For_i(0, loop_reps, 1):
                    _emit_body(nc, mybir, q_d, s_d, o_d, ones, rall, pools,
                               parts=parts, split=split)

    nc.compile()
    return nc


def _to_bf16(x):
    import ml_dtypes

    return np.ascontiguousarray(x, dtype=np.float32).astype(ml_dtypes.bfloat16)


def _prep(query_local, support_local):
    q = _to_bf16(query_local).reshape(N_CORES, BPC, D, NQ)
    s = _to_bf16(support_local).reshape(N_CORES, BPC, D, NS)
    return q, s


def kernel(query_local, support_local):
    from concourse.bass_utils import run_bass_kernel_spmd

    if "nc" not in _cache:
        _cache["nc"] = _build()
    nc = _cache["nc"]

    q, s = _prep(query_local, support_local)
    in_maps = [{"q": q[c], "s": s[c]} for c in range(N_CORES)]
    res = run_bass_kernel_spmd(nc, in_maps, list(range(N_CORES)))
    outs = [np.asarray(res.results[c]["o"]).reshape(BPC) for c in range(N_CORES)]
    return np.concatenate(outs, axis=0)
